# revision 32
# baseline (speedup 1.0000x reference)
"""Trainium2 Bass kernel for nn_Discriminator (down-projection + 16-step LSTM + head).

Computation (per reference):
    x: [512, 16, 10001] fp32
    xa = x[:, :, :10000] @ W_down                      # [B, T, 128]
    xc_t = concat([xa_t, xw_t], -1)                    # per step, [B, 129]
    LSTM over T=16 steps, H=512, forget bias +1:
        gates = [xc_t, h] @ W_cell + b_cell            # [B, 2048] = [i|c|f|o]
        c = c*sig(f+1) + sig(i)*tanh(c_)
        h = sig(o)*tanh(c)
    pred = h @ W_out + b_out                           # [B, 1]

Sharding: pure data-parallel over batch, 64 rows/core on 8 cores. No collectives.

Per-core design (v2, restructured from the ~175us baseline):
  - x is cast to bf16 and pre-transposed/tiled on the host so the PE
    contraction dim (n) is on partitions and every down-projection slab DMA
    is one contiguous row-block per partition.
  - The down-projection accumulates xa^T in 4 column chunks (4 LSTM steps
    each); chunk 0 runs up front (DMA-paced, with HAM warm-keeper matmuls),
    the rest are emitted inside the LSTM steps' PE-idle windows.
  - Each LSTM gate gets its OWN [128, 256] PSUM tile: batch (64) x 2
    H-halves stacked on partitions, so every activation / vector op runs at
    the full 128-partition width.  Gate matmuls are ordered c~, i, f, o and
    each gate's accumulation group closes separately, so tanh(c~) starts
    after only a quarter of the recurrent matmuls.
  - Native Tanh for c~ / c and Sigmoid for i, f, o (both live in the same
    activation table set, loaded once).  No host-side gate folding beyond
    the +1 forget bias and a gate-column permutation.
  - Cell state and all activation outputs are bf16 (2x DVE throughput);
    cell update is 3 DVE ops (m2=tanh(c~)*sig(i), m1=sig(f)*c_prev,
    c=m1+m2), then tanh(c) on ACT and h=tanh(c)*sig(o) on DVE.
  - h is PE-transposed (4x [64,128]) into one PSUM tile and copied once to
    form the next step's lhsT; next step's x-path gate contributions are
    pre-accumulated into fresh PSUM tiles during the current step's
    activation chain.
"""

import numpy as np
from contextlib import ExitStack

NCORES = 8
B = 512
BC = B // NCORES          # 64 batch rows per core
T = 16
BT = BC * T               # 1024
N = 10000
KT = 79                   # ceil(10000/128)
NPAD = KT * 128           # 10112
HIN = 128
H = 512
G4 = 4 * H                # 2048
GH = 256                  # gate half-width (columns per gate tile)
DP_CHUNKS = 4             # down-projection output chunks (t-blocks)
CCOLS = BT // DP_CHUNKS   # columns per chunk (256)

_CACHE = {}


def _build_module(sim=False):
    import concourse.bass as bass  # noqa: F401
    import concourse.bacc as bacc
    import concourse.tile as tile
    import concourse.mybir as mybir

    AF = mybir.ActivationFunctionType
    f32 = mybir.dt.float32
    bf16 = mybir.dt.bfloat16

    if sim:
        nc = bacc.Bacc(None, target_bir_lowering=False, debug=True)
    else:
        nc = bacc.Bacc("TRN2")

    # x pre-tiled on host to [128, DP_CHUNKS, KT, CCOLS] (flattened along the
    # free dim) so every slab DMA is one contiguous row-block per partition
    xT = nc.declare_dram_parameter("xT", [128, DP_CHUNKS * KT * CCOLS], bf16, isOutput=False)
    xw = nc.declare_dram_parameter("xw", [2, BT], bf16, isOutput=False)
    Wd = nc.declare_dram_parameter("Wd", [128, KT * 128], bf16, isOutput=False)
    Wxa = nc.declare_dram_parameter("Wxa", [128, G4], bf16, isOutput=False)
    Wxwb = nc.declare_dram_parameter("Wxwb", [2, G4], bf16, isOutput=False)
    Wh = nc.declare_dram_parameter("Wh", [128, 4 * G4], bf16, isOutput=False)
    Wo = nc.declare_dram_parameter("Wo", [128, 4], bf16, isOutput=False)
    bout = nc.declare_dram_parameter("bout", [BC, 1], f32, isOutput=False)
    ident = nc.declare_dram_parameter("ident", [128, 128], bf16, isOutput=False)
    pred = nc.declare_dram_parameter("pred", [BC, 1], f32, isOutput=True)

    # slab granularity: KG k-tiles per DMA (bigger transfers, fewer issues)
    KG = 8
    NSLAB = (KT + KG - 1) // KG  # 10 (last slab holds 7 k-tiles)

    with ExitStack() as ctx:
        tc = ctx.enter_context(tile.TileContext(nc))
        singles = ctx.enter_context(tc.tile_pool(name="singles", bufs=1))
        slabs = ctx.enter_context(tc.tile_pool(name="slabs", bufs=6))
        work = ctx.enter_context(tc.tile_pool(name="work", bufs=2))
        state = ctx.enter_context(tc.tile_pool(name="state", bufs=2))
        dpp = ctx.enter_context(tc.tile_pool(name="dpp", bufs=2, space="PSUM"))
        gp = ctx.enter_context(tc.tile_pool(name="gp", bufs=1, space="PSUM"))
        tp = ctx.enter_context(tc.tile_pool(name="tp", bufs=2, space="PSUM"))

        # W_down first (the dp stream's only prerequisite), split across DMA
        # lanes so the first k-tiles land quickly
        Wd_sb = singles.tile([128, KT * 128], bf16)
        wd_step = 10 * 128
        for o in range(0, KT * 128, wd_step):
            e = min(o + wd_step, KT * 128)
            nc.scalar.dma_start(Wd_sb[:, o:e], Wd[:, o:e])

        # warm-up burst: ~4us of back-to-back matmuls on a memset tile (no
        # DMA dependency) so the HAM clock gate reaches 2.4 GHz before the
        # first dp slab lands; otherwise the whole DMA-paced chunk-0 stream
        # runs at the cold 1.2 GHz rate
        wtile = singles.tile([128, 256], bf16)
        nc.vector.memset(wtile[:], 1.0)
        wp = tp.tile([128, 128], f32, tag="tp")
        NWARM = 24
        for w in range(NWARM):
            nc.tensor.matmul(wp[:], wtile[:, 0:128], wtile[:, 128:256],
                             start=(w == 0), stop=(w == NWARM - 1))

        # xa^T, one tile per chunk so the LSTM's dependency is per-chunk
        xaT_sb = [singles.tile([128, CCOLS], bf16, name=f"xaT{c}") for c in range(DP_CHUNKS)]

        def dp_slab(c, ps, s):
            k0 = s * KG
            nk = min(KG, KT - k0)
            sl = slabs.tile([128, KG, CCOLS], bf16, tag="slab")
            off = (c * KT + k0) * CCOLS
            nc.sync.dma_start(sl[:, :nk, :],
                              xT[:, off:off + nk * CCOLS].rearrange("p (t c) -> p t c", c=CCOLS))
            for j in range(nk):
                k = k0 + j
                nc.tensor.matmul(ps[:], Wd_sb[:, k * 128:(k + 1) * 128], sl[:, j, :],
                                 start=(k == 0), stop=(k == KT - 1))

        # down-projection chunk-0 stream emitted first; LSTM weights after
        ps0 = dpp.tile([128, CCOLS], f32, tag="dp", name="ps0")
        for s in range(NSLAB):
            dp_slab(0, ps0, s)
        nc.scalar.copy(xaT_sb[0][:], ps0[:])

        # LSTM weights + small tensors.  The prologue is HBM-bandwidth
        # bound (chunk 0 + weights compete); order by first use and defer
        # Wh (2 MB, first needed at step 1's recurrent matmuls) to the
        # very end, split per k-tile so the first piece lands early.
        Wxa_sb = singles.tile([128, G4], bf16)
        nc.scalar.dma_start(Wxa_sb[:], Wxa[:])
        Wxwb_sb = singles.tile([2, G4], bf16)
        nc.scalar.dma_start(Wxwb_sb[:], Wxwb[:])
        xw_sb = singles.tile([2, BT], bf16)
        nc.scalar.dma_start(xw_sb[:], xw[:])
        bout_sb = singles.tile([BC, 1], f32)
        nc.scalar.dma_start(bout_sb[:], bout[:])
        id_sb = singles.tile([128, 128], bf16)
        nc.scalar.dma_start(id_sb[:], ident[:])
        Wo_sb = singles.tile([128, 4], bf16)
        nc.scalar.dma_start(Wo_sb[:], Wo[:])
        Wh_sb = singles.tile([128, 4 * G4], bf16)
        for k in range(4):
            nc.scalar.dma_start(Wh_sb[:, k * G4:(k + 1) * G4],
                                Wh[:, k * G4:(k + 1) * G4])

        hT_prev = None
        c_prev = None
        g_next = None  # list of 4 psum gate tiles for the NEXT step

        def gslice(tiles, g, hh=None):
            # one PSUM bank per gate (single-buffered: the next step's
            # pre-accumulation reaches the PE long after this step's single
            # ACT read of each gate, so no double-buffering is needed and
            # each gate's accumulation group closes independently)
            tl = tiles[g]
            if hh is None:
                return tl[:, :]
            return tl[hh * 64:(hh + 1) * 64, :]

        def preacc(t, close=False):
            """Allocate the 4 gate tiles for step t and pre-accumulate ALL
            gates' h-independent contributions (xa @ Wxa + [xw,1] @ Wxwb) in
            the previous step's PE shadow.  start=True on each gate's xa MM
            opens its group; close=True ends it at the xw MM (t=0, which
            has no h terms).  skip_group_check: CoreSim's zero-region
            conflict checker mis-resolves partition-base>0 PSUM offsets
            (aliases other banks); correctness holds by PE program order."""
            tiles = [gp.tile([128, GH], f32, tag=f"g{g}", name=f"g{t}_{g}")
                     for g in range(4)]
            xa_lh = xaT_sb[t * BC // CCOLS][:, (t * BC) % CCOLS:(t * BC) % CCOLS + BC]
            xw_lh = xw_sb[:, t * BC:(t + 1) * BC]
            for g in (0, 1, 2, 3):
                for ki, lh in enumerate((xa_lh, xw_lh)):
                    st = ki == 0
                    sp = close and ki == 1
                    rh_all = Wxa_sb if ki == 0 else Wxwb_sb
                    for hh in range(2):
                        rh = rh_all[:, (g * 2 + hh) * GH:(g * 2 + hh + 1) * GH]
                        nc.tensor.matmul(gslice(tiles, g, hh), lh, rh,
                                         start=st, stop=sp,
                                         skip_group_check=True)
            return tiles

        # dp slab queue for chunks 1-3, drained 3 per step inside the LSTM;
        # chunk copies go on the vector engine so they never delay the
        # activation chain on the scalar engine
        slab_q = [(c, s) for c in range(1, DP_CHUNKS) for s in range(NSLAB)]
        cur_ps = [None] * DP_CHUNKS

        def dp_fill(n):
            for _ in range(min(n, len(slab_q))):
                c, s = slab_q.pop(0)
                if s == 0:
                    cur_ps[c] = dpp.tile([128, CCOLS], f32, tag="dp", name=f"dps{c}")
                dp_slab(c, cur_ps[c], s)
                if s == NSLAB - 1:
                    nc.vector.tensor_copy(xaT_sb[c][:], cur_ps[c][:])

        def hmms(gt, g):
            # recurrent gate matmuls for one gate: 4 k-tiles, the two
            # H-halves on the two output-partition halves (concurrent PE
            # column groups); k==3 closes the gate's accumulation group
            for k in range(4):
                lh = hT_prev[:, k % 2, (k // 2) * BC:(k // 2) * BC + BC]
                sp = k == 3
                for hh in range(2):
                    rh = Wh_sb[:, k * G4 + (g * 2 + hh) * GH:
                               k * G4 + (g * 2 + hh + 1) * GH]
                    nc.tensor.matmul(gslice(gt, g, hh), lh, rh,
                                     start=False, stop=sp,
                                     skip_group_check=True)

        def lstm_step(t):
            nonlocal hT_prev, c_prev, g_next
            gt = g_next
            # gate order c~, i, f, o: bank A (c~,i) closes first so tanh /
            # sig(i) start while bank B's matmuls still stream
            if hT_prev is not None:
                for g in (0, 1, 2, 3):
                    hmms(gt, g)

            # dp slabs + next step's pre-accumulation in the PE shadow of
            # this step's activation chain.  Fills are split so the
            # transposes aren't queued behind the whole dp batch; dp_fill
            # must be emitted before a preacc that reads its xaT chunk
            # (Tile's dependency tracking is trace-order-based).  Step 0
            # has no recurrent matmuls, so its chain shadow absorbs an
            # extra slab.
            dp_fill(2 if t == 0 else 1)
            if t + 1 < T:
                g_next = preacc(t + 1)
            dp_fill(1)

            # activation chain: T=tanh(c~), I/F/O=sigmoid, all [128,256] bf16
            Tt = work.tile([128, GH], bf16, tag="T")
            nc.scalar.activation(Tt[:], gslice(gt, 0), AF.Tanh)
            It = work.tile([128, GH], bf16, tag="I")
            nc.scalar.activation(It[:], gslice(gt, 1), AF.Sigmoid)
            if t > 0:
                Ft = work.tile([128, GH], bf16, tag="F")
                nc.scalar.activation(Ft[:], gslice(gt, 2), AF.Sigmoid)
            Ot = work.tile([128, GH], bf16, tag="O")
            nc.scalar.activation(Ot[:], gslice(gt, 3), AF.Sigmoid)

            # cell update on DVE (bf16 -> 2x mode):
            #   m2 = tanh(c~)*sig(i); m1 = sig(f)*c_prev; c = m1 + m2
            m2 = work.tile([128, GH], bf16, tag="m2")
            nc.vector.tensor_mul(m2[:], Tt[:], It[:])
            if t > 0:
                m1 = work.tile([128, GH], bf16, tag="m1")
                nc.vector.tensor_mul(m1[:], Ft[:], c_prev[:])
                c_new = work.tile([128, GH], bf16, tag="c")
                nc.vector.tensor_add(c_new[:], m1[:], m2[:])
            else:
                c_new = m2
            c_prev = c_new

            # h = tanh(c) * sig(o)
            CN = work.tile([128, GH], bf16, tag="CN")
            nc.scalar.activation(CN[:], c_new[:], AF.Tanh)
            h = work.tile([128, GH], bf16, tag="h")
            nc.vector.tensor_mul(h[:], CN[:], Ot[:])

            # transpose h into the next step's lhsT via 2 full-partition PE
            # transposes (stationary = h[:, j*128:(j+1)*128], uniform base —
            # mixing 64-row stationaries at bases 0/64 aborts on HW), then a
            # single copy.  tps[p, j, q] = h[q, j*128+p], so the k-tile lhsT
            # is hT[:, k%2, (k//2)*64 + b].
            hT = state.tile([128, 2, 128], bf16, tag="hT")
            tps = tp.tile([128, 2, 128], bf16, tag="tp")
            for j in range(2):
                nc.tensor.transpose(tps[:, j, :], h[:, j * 128:(j + 1) * 128], id_sb[:])
            nc.vector.tensor_copy(hT[:], tps[:])
            hT_prev = hT
            # one more dp slab while the next step waits on the hT copy
            dp_fill(1)

        g_next = preacc(0, close=True)
        for t in range(T):
            lstm_step(t)

        # output head: pred = h_T @ W_out + b_out
        ps_p = tp.tile([BC, 1], f32, tag="tp")
        for k in range(4):
            nc.tensor.matmul(ps_p[:], hT_prev[:, k % 2, (k // 2) * BC:(k // 2) * BC + BC],
                             Wo_sb[:, k:k + 1], start=(k == 0), stop=(k == 3))
        out_sb = singles.tile([BC, 1], f32)
        nc.scalar.activation(out_sb[:], ps_p[:], AF.Identity, bias=bout_sb[:])
        nc.sync.dma_start(pred[:], out_sb[:])

    if sim:
        nc.compile()
    else:
        nc.finalize()
    return nc


def _get_module():
    if "m" not in _CACHE:
        _CACHE["m"] = _build_module()
    return _CACHE["m"]


def _prep_inputs(x, W_down, W_cell, b_cell, W_out, b_out):
    import ml_dtypes
    bf16 = ml_dtypes.bfloat16
    x = np.asarray(x, dtype=np.float32)
    W_down = np.asarray(W_down, dtype=np.float32)
    W_cell = np.asarray(W_cell, dtype=np.float32)
    b_cell = np.asarray(b_cell, dtype=np.float32)
    W_out = np.asarray(W_out, dtype=np.float32)
    b_out = np.asarray(b_out, dtype=np.float32)

    # shared tensors
    Wd_pad = np.zeros((NPAD, HIN), dtype=np.float32)
    Wd_pad[:N] = W_down
    # [NPAD, 128] -> per-k-tile layout [128, KT*128] (col block k = k-tile)
    Wd_host = np.ascontiguousarray(
        Wd_pad.reshape(KT, 128, HIN).transpose(1, 0, 2).reshape(128, KT * HIN)
    ).astype(bf16)

    # forget-gate bias +1, then permute gate columns [i|c|f|o] ->
    # [(c~,h0)(c~,h1)(i,h0)(i,h1)(f,h0)(f,h1)(o,h0)(o,h1)] where h0/h1 are
    # the 256-column halves of each gate (stacked on output partitions)
    b_mod = b_cell.copy()
    b_mod[1024:1536] += 1.0
    perm = np.concatenate([np.arange(base + hh * GH, base + (hh + 1) * GH)
                           for base in (512, 0, 1024, 1536) for hh in (0, 1)])
    Wmod = W_cell[:, perm]
    b_mod = b_mod[perm]
    Wxa_host = np.ascontiguousarray(Wmod[0:HIN]).astype(bf16)            # [128, 2048]
    Wxwb_host = np.stack([Wmod[HIN], b_mod]).astype(bf16)                # [2, 2048]
    Wh_host = np.ascontiguousarray(
        Wmod[HIN + 1:].reshape(4, 128, G4).transpose(1, 0, 2).reshape(128, 4 * G4)
    ).astype(bf16)                                                       # [128, 4*2048]
    Wo_host = np.ascontiguousarray(W_out.reshape(4, 128).T).astype(bf16)  # [128, 4]
    bout_host = np.full((BC, 1), float(b_out[0]), dtype=np.float32)
    id_host = np.eye(128, dtype=np.float32).astype(bf16)

    in_maps = []
    for i in range(NCORES):
        xs = x[i * BC:(i + 1) * BC]                       # [64, 16, 10001]
        # xT: [NPAD, 1024], column index = t*64 + b (t-major)
        xT_host = np.zeros((NPAD, BT), dtype=bf16)
        xT_host[:N] = xs[:, :, :N].transpose(2, 1, 0).reshape(N, BT).astype(bf16)
        # re-tile to [128, DP_CHUNKS, KT, CCOLS] flattened on the free dim so
        # each (chunk, k-group) slab is contiguous per partition:
        # xt4[p, c, k, j] = xT[k*128 + p, c*CCOLS + j]
        xt4 = xT_host.reshape(KT, 128, DP_CHUNKS, CCOLS).transpose(1, 2, 0, 3)
        xT_host = np.ascontiguousarray(xt4).reshape(128, DP_CHUNKS * KT * CCOLS)
        xw_host = np.empty((2, BT), dtype=bf16)
        xw_host[0] = xs[:, :, N].T.reshape(BT).astype(bf16)
        xw_host[1] = np.ones(BT, dtype=np.float32).astype(bf16)
        in_maps.append({
            "xT": xT_host,
            "xw": xw_host,
            "Wd": Wd_host,
            "Wxa": Wxa_host,
            "Wxwb": Wxwb_host,
            "Wh": Wh_host,
            "Wo": Wo_host,
            "bout": bout_host,
            "ident": id_host,
        })
    return in_maps


def run(trace=False, **inputs):
    from concourse.bass_utils import run_bass_kernel_spmd

    nc = _get_module()
    in_maps = _prep_inputs(**inputs)
    res = run_bass_kernel_spmd(nc, in_maps, list(range(NCORES)), trace=trace)
    pred = np.concatenate([res.results[i]["pred"] for i in range(NCORES)], axis=0)
    return pred.astype(np.float32), res


def kernel(**inputs):
    pred, _ = run(trace=False, **inputs)
    return pred
